# revision 1
# baseline (speedup 1.0000x reference)
"""Trainium2 Bass kernel for nn_AbstractAtt (MLB-style attention + fusion + classifier).

Data-parallel over 8 NeuronCores: batch 128 -> 16 samples/core, weights replicated.

Per-core pipeline (matmuls in fp32r = TF32-class; Wf/Wc streamed as bf16):
  x_v   = tanh(v^T @ Wv + bv)            [DA, S] orientation, PE + ACT(bias fused)
  x_att = tanh(x_v * x_q[b])             ACT with per-partition scale (x_q^T col)
  scores= x_att^T @ Wa                   [G, S] PSUM accumulation over DA tiles
  att   = exp(scores + ba); row-sums via ACT accum_out; normalization folded
          into the pooled output (softmax denominator applied post-pooling)
  pool  = e^T @ v^T                      needs v^T: PE-transposed per sample
  xv    = tanh(v_att @ Wf[g] + bf)       glimpse-packed lhsT (vaT columns g*16+b)
  x     = tanh(xv * xq);  out = x @ Wc + bc

DMA-count discipline: the HWDGE ring costs ~0.6us serial per dma_start, so
weight/input streams are merged into multi-k-tile transfers.
"""

import os

import ml_dtypes
import numpy as np

import concourse.bass as bass
import concourse.mybir as mybir
import concourse.tile as tile
from concourse import bacc
from concourse.bass_utils import run_bass_kernel_spmd
from concourse.masks import make_identity

F32 = mybir.dt.float32
F32R = mybir.dt.float32r
BF16 = mybir.dt.bfloat16
AF = mybir.ActivationFunctionType
KVARIANT = os.environ.get("KVARIANT", "full")  # timing probes: full|pairs|notrans
NPAIR_OVR = int(os.environ.get("NPAIRS", "0")) or None

# problem constants (hardcoded per contract)
B, DV, W, H = 128, 2048, 14, 14
S = W * H            # 196
DQ = 2048
DA = 1200
G = 4
DH = 2048
DHG = DH // G        # 512
NANS = 3000
NCORES = 8
BPC = B // NCORES    # 16 samples per core
NPAIR = BPC // 2     # 8 pairs

NK = DV // 128       # 16 k-tiles over DV (== DQ // 128)
KM = 4               # k-tiles merged per DMA / per SBUF tile
NKM = NK // KM       # 4 merged groups
DA_TILES = [(m * 128, min(128, DA - m * 128)) for m in range((DA + 127) // 128)]
S0, S1 = 128, S - 128          # S split 128 + 68
S2 = 2 * S                     # 392 columns per pair
NANS_TILES = [(j * 500, 500) for j in range(6)]
XQF_TILES = [(j * 256, 256) for j in range(8)]


def build_module(reps: int = 1) -> bacc.Bacc:
    nc = bacc.Bacc("TRN2", target_bir_lowering=False, debug=False)

    v = nc.dram_tensor("v", [BPC, DV, S], F32, kind="ExternalInput").ap()
    q = nc.dram_tensor("q", [BPC, DQ], F32, kind="ExternalInput").ap()
    wv = nc.dram_tensor("wv", [DV, DA], F32, kind="ExternalInput").ap()
    bv = nc.dram_tensor("bv", [DA, 1], F32, kind="ExternalInput").ap()
    wq = nc.dram_tensor("wq", [DQ, DA], F32, kind="ExternalInput").ap()
    bq = nc.dram_tensor("bq", [DA, 1], F32, kind="ExternalInput").ap()
    wa = nc.dram_tensor("wa", [DA, G], F32, kind="ExternalInput").ap()
    ba = nc.dram_tensor("ba", [G, 1], F32, kind="ExternalInput").ap()
    wf = nc.dram_tensor("wf", [G, DV, DHG], BF16, kind="ExternalInput").ap()
    bf = nc.dram_tensor("bf", [1, DH], F32, kind="ExternalInput").ap()
    wqf = nc.dram_tensor("wqf", [DQ, DH], F32, kind="ExternalInput").ap()
    bqf = nc.dram_tensor("bqf", [1, DH], F32, kind="ExternalInput").ap()
    wc = nc.dram_tensor("wc", [DQ, NANS], BF16, kind="ExternalInput").ap()
    bc = nc.dram_tensor("bc", [1, NANS], F32, kind="ExternalInput").ap()
    out = nc.dram_tensor("out", [BPC, NANS], F32, kind="ExternalOutput").ap()

    with tile.TileContext(nc) as tc:
        if reps > 4:
            # device-side loop for timing: constant code size, R iterations
            with tc.For_i(0, reps, 1):
                emit_core(nc, tc, v, q, wv, bv, wq, bq, wa, ba, wf, bf, wqf,
                          bqf, wc, bc, out)
        else:
            for rep in range(reps):
                emit_core(nc, tc, v, q, wv, bv, wq, bq, wa, ba, wf, bf, wqf,
                          bqf, wc, bc, out)
    nc.compile()
    return nc


def emit_core(nc, tc, v, q, wv, bv, wq, bq, wa, ba, wf, bf, wqf, bqf, wc, bc, out):
    from contextlib import ExitStack

    ctx = ExitStack()
    with ctx:
        # ---------------- persistent pools ----------------
        const_pool = ctx.enter_context(tc.tile_pool(name="const", bufs=1))
        wv_pool = ctx.enter_context(tc.tile_pool(name="wvp", bufs=1))

        ident = const_pool.tile([128, 128], F32)
        make_identity(nc, ident[:])
        ident_r = const_pool.tile([128, 128], F32R)
        nc.vector.tensor_copy(ident_r[:], ident[:])
        ones_f = const_pool.tile([1, 16], F32)
        nc.gpsimd.memset(ones_f[:], 1.0)
        ones = const_pool.tile([1, 16], F32R)
        nc.vector.tensor_copy(ones[:], ones_f[:])

        # per-partition bias tiles, packed loads (col m = DA tile m)
        bv_sb = const_pool.tile([128, len(DA_TILES)], F32)
        bq_sb = const_pool.tile([128, len(DA_TILES)], F32)
        nc.sync.dma_start(bv_sb[:, :9], bv[0:1152, 0].rearrange("(m p) -> p m", p=128))
        nc.sync.dma_start(bv_sb[:48, 9:10], bv[1152:1200, :])
        nc.sync.dma_start(bq_sb[:, :9], bq[0:1152, 0].rearrange("(m p) -> p m", p=128))
        nc.sync.dma_start(bq_sb[:48, 9:10], bq[1152:1200, :])
        ba_sb = const_pool.tile([G, 1], F32)
        nc.sync.dma_start(ba_sb[:], ba[:])
        # wa packed: [128, 40] f32r, cols m*4..m*4+4 = Wa rows m*128..+128
        wa_sb = const_pool.tile([128, G * len(DA_TILES)], F32R)
        nc.sync.dma_start(
            wa_sb[:, :36].rearrange("p (m g) -> p m g", g=G),
            wa[0:1152, :].rearrange("(m p) g -> p m g", p=128).bitcast(F32R))
        nc.sync.dma_start(wa_sb[:48, 36:40], wa[1152:1200, :].bitcast(F32R))
        bqf_sb = const_pool.tile([1, DH], F32R)
        nc.sync.dma_start(bqf_sb[:], bqf[:].bitcast(F32R))

        # ---------------- pre-phase: q^T, x_q^T ----------------
        with tc.tile_pool(name="pre", bufs=1) as pre, \
             tc.tile_pool(name="pre_ps", bufs=1, space="PSUM") as pre_ps:
            q_sb = pre.tile([BPC, DQ], F32)
            nc.sync.dma_start(q_sb[:], q[:])
            # qT: [DQ(k-tiles), BPC] in f32r, one tile [128, NK*16]
            qT = const_pool.tile([128, NK * BPC], F32R)
            for k in range(NK):
                p = pre_ps.tile([128, BPC], F32, tag="qt", bufs=2)
                nc.tensor.transpose(p[:], q_sb[:, k * 128:(k + 1) * 128],
                                    ident[:BPC, :BPC])
                nc.vector.tensor_copy(qT[:, k * BPC:(k + 1) * BPC], p[:])

            # x_q_lin = q @ Wq  ([BPC, DA]); Wq streamed 4-k-merged per chunk
            xq_lin = pre.tile([BPC, DA], F32)
            for j, (n0, nw) in enumerate([(0, 400), (400, 400), (800, 400)]):
                pj = pre_ps.tile([BPC, nw], F32, tag=f"xq{j}")
                for kk in range(NKM):
                    wt = pre.tile([128, KM * nw], F32R, tag="wqs", bufs=2)
                    nc.sync.dma_start(
                        wt[:].rearrange("p (k n) -> p k n", k=KM),
                        wq[kk * KM * 128:(kk + 1) * KM * 128, n0:n0 + nw]
                        .rearrange("(k c) n -> c k n", k=KM).bitcast(F32R))
                    for ki in range(KM):
                        k = kk * KM + ki
                        nc.tensor.matmul(pj[:], qT[:, k * BPC:(k + 1) * BPC],
                                         wt[:, ki * nw:(ki + 1) * nw],
                                         start=(k == 0), stop=(k == NK - 1))
                nc.vector.tensor_copy(xq_lin[:, n0:n0 + nw], pj[:])

            # x_qT[m] = tanh(xq_lin^T + bq) per DA tile  -> [mw, BPC] f32
            xqT = const_pool.tile([128, len(DA_TILES) * BPC], F32)
            for m, (m0, mw) in enumerate(DA_TILES):
                p = pre_ps.tile([128, BPC], F32, tag="qt", bufs=2)
                nc.tensor.transpose(p[:mw, :], xq_lin[:, m0:m0 + mw],
                                    ident[:BPC, :BPC])
                nc.scalar.activation(xqT[:mw, m * BPC:(m + 1) * BPC], p[:mw, :],
                                     AF.Tanh, bias=bq_sb[:mw, m:m + 1])

        # resident Wv tiles (f32r), 4-k-merged: wv_sb[kk][:, ki*DA + d]
        wv_sb = []
        for kk in range(NKM):
            t = wv_pool.tile([128, KM * DA], F32R, tag=f"wv{kk}")
            nc.sync.dma_start(
                t[:].rearrange("p (k d) -> p k d", k=KM),
                wv[kk * KM * 128:(kk + 1) * KM * 128, :]
                .rearrange("(k c) d -> c k d", k=KM).bitcast(F32R))
            wv_sb.append(t)

        def wv_lhsT(k, m0, mw):
            return wv_sb[k // KM][:, (k % KM) * DA + m0:(k % KM) * DA + m0 + mw]

        # xqf accumulator [BPC, DH], built in chunks interleaved with pair loop
        xqf_sb = const_pool.tile([BPC, DH], F32)
        # v_att collection [4*BPC, DV] (partition = 4*b + g)
        vatt_sb = const_pool.tile([4 * BPC, DV], F32)

        # ---------------- pair loop ----------------
        # wf stream pool hoisted so its slots exist early -> prefetch in pairs
        wfp = ctx.enter_context(tc.tile_pool(name="wfp", bufs=1))
        with tc.tile_pool(name="pl", bufs=1) as pl, \
             tc.tile_pool(name="pl_ps", bufs=1, space="PSUM") as pl_ps:

            def xqf_chunk(j):
                n0, nw = XQF_TILES[j]
                pj = pl_ps.tile([BPC, nw], F32, tag="pqf", bufs=1)
                for kk in range(NKM):
                    wt = pl.tile([128, KM * nw], F32R, tag="wqf", bufs=3)
                    nc.sync.dma_start(
                        wt[:].rearrange("p (k n) -> p k n", k=KM),
                        wqf[kk * KM * 128:(kk + 1) * KM * 128, n0:n0 + nw]
                        .rearrange("(k c) n -> c k n", k=KM).bitcast(F32R))
                    for ki in range(KM):
                        k = kk * KM + ki
                        nc.tensor.matmul(pj[:], qT[:, k * BPC:(k + 1) * BPC],
                                         wt[:, ki * nw:(ki + 1) * nw],
                                         start=(k == 0), stop=False,
                                         skip_group_check=True)
                nc.tensor.matmul(pj[:], ones[:, :BPC], bqf_sb[:, n0:n0 + nw],
                                 start=False, stop=True, skip_group_check=True)
                nc.scalar.activation(xqf_sb[:, n0:n0 + nw], pj[:], AF.Tanh)

            for pair in range(NPAIR_OVR or NPAIR):
                b0 = pair * 2
                # v pair tiles, 4-k-merged: v_sb[kk][:, (ki, b, s)]
                v_sb = []
                for kk in range(NKM):
                    t = pl.tile([128, KM * S2], F32R, tag=f"v{kk}", bufs=2)
                    tv = t[:].rearrange("p (k b s) -> p k b s", k=KM, b=2)
                    for s in range(2):
                        nc.sync.dma_start(
                            tv[:, :, s, :],
                            v[b0 + s, kk * KM * 128:(kk + 1) * KM * 128, :]
                            .rearrange("(k c) s -> c k s", k=KM).bitcast(F32R))
                    v_sb.append(t)

                def v_rhs(k, lo=0, width=S2):
                    return v_sb[k // KM][:, (k % KM) * S2 + lo:
                                         (k % KM) * S2 + lo + width]

                p_sc = pl_ps.tile([G, S2], F32, tag="psc", bufs=1)
                for m, (m0, mw) in enumerate(DA_TILES):
                    pm = pl_ps.tile([128, S2], F32, tag="pmain", bufs=2)
                    for k in range(NK):
                        nc.tensor.matmul(pm[:mw, :], wv_lhsT(k, m0, mw),
                                         v_rhs(k), start=(k == 0),
                                         stop=(k == NK - 1),
                                         skip_group_check=True)
                    # x_v = tanh(mm + bv)
                    xv_t = pl.tile([128, S2], F32, tag="xv", bufs=2)
                    nc.scalar.activation(xv_t[:mw, :], pm[:mw, :], AF.Tanh,
                                         bias=bv_sb[:mw, m:m + 1])
                    # x_att = tanh(x_v * xq[b])  per sample
                    xa_t = pl.tile([128, S2], F32R, tag="xa", bufs=2)
                    for s in range(2):
                        nc.scalar.activation(
                            xa_t[:mw, s * S:(s + 1) * S],
                            xv_t[:mw, s * S:(s + 1) * S], AF.Tanh,
                            scale=xqT[:mw, m * BPC + b0 + s:m * BPC + b0 + s + 1])
                    # scores accumulation
                    nc.tensor.matmul(p_sc[:], wa_sb[:mw, m * G:(m + 1) * G],
                                     xa_t[:mw, :],
                                     start=(m == 0), stop=(m == len(DA_TILES) - 1),
                                     skip_group_check=True)

                # att = exp(scores + ba), with per-sample row sums
                e_sb = pl.tile([G, S2], F32, tag="e", bufs=1)
                esum = pl.tile([G, 2], F32, tag="esum", bufs=2)
                for s in range(2):
                    nc.scalar.activation(e_sb[:, s * S:(s + 1) * S],
                                         p_sc[:, s * S:(s + 1) * S], AF.Exp,
                                         bias=ba_sb[:], accum_out=esum[:, s:s + 1])
                recip = pl.tile([G, 2], F32, tag="recip", bufs=2)
                nc.vector.reciprocal(recip[:], esum[:])

                for s in range(2):
                    # e^T tiles for this sample: [128,4] + [68,4]
                    eT0 = pl.tile([S0, G], F32R, tag="eT0", bufs=2)
                    eT1 = pl.tile([S1, G], F32R, tag="eT1", bufs=2)
                    pt0 = pl_ps.tile([S0, G], F32, tag="peT", bufs=1)
                    nc.tensor.transpose(pt0[:], e_sb[:, s * S:s * S + S0],
                                        ident[:G, :G])
                    nc.vector.tensor_copy(eT0[:], pt0[:])
                    pt1 = pl_ps.tile([S0, G], F32, tag="peT", bufs=1)
                    nc.tensor.transpose(pt1[:S1, :], e_sb[:, s * S + S0:(s + 1) * S],
                                        ident[:G, :G])
                    nc.vector.tensor_copy(eT1[:], pt1[:S1, :])

                    # v^T tiles for this sample: [128, DV] and [68, DV]
                    if KVARIANT == "notrans":
                        vT0 = vT1 = None
                    else:
                        vT0 = pl.tile([S0, DV], F32R, tag="vT0", bufs=1)
                    if KVARIANT != "notrans":
                        vT1 = pl.tile([S1, DV], F32R, tag="vT1", bufs=1)
                    for k in range(NK if KVARIANT != "notrans" else 0):
                        pv0 = pl_ps.tile([S0, 128], F32R, tag="pvT", bufs=2)
                        nc.tensor.transpose(pv0[:], v_rhs(k, s * S, S0), ident_r[:])
                        nc.vector.tensor_copy(
                            vT0[:, k * 128:(k + 1) * 128], pv0[:].bitcast(F32))
                        pv1 = pl_ps.tile([S0, 128], F32R, tag="pvT", bufs=2)
                        nc.tensor.transpose(pv1[:S1, :], v_rhs(k, s * S + S0, S1),
                                            ident_r[:])
                        nc.vector.tensor_copy(
                            vT1[:, k * 128:(k + 1) * 128], pv1[:S1, :].bitcast(F32))

                    # pooling: U[g, c] = e^T @ v^T; normalize into tmp; 1 DMA
                    tmp = pl.tile([G, DV], F32, tag="ptmp", bufs=1)
                    for cchunk in range(DV // 512):
                        c0 = cchunk * 512
                        pp = pl_ps.tile([G, 512], F32, tag="ppool", bufs=1)
                        if KVARIANT == "notrans":
                            nc.tensor.matmul(pp[:], eT0[:], v_sb[cchunk][:, :512],
                                             start=True, stop=False)
                            nc.tensor.matmul(pp[:], eT1[:], v_sb[cchunk][:68, :512],
                                             start=False, stop=True)
                        else:
                            nc.tensor.matmul(pp[:], eT0[:], vT0[:, c0:c0 + 512],
                                             start=True, stop=False)
                            nc.tensor.matmul(pp[:], eT1[:], vT1[:, c0:c0 + 512],
                                             start=False, stop=True)
                        nc.vector.tensor_scalar_mul(tmp[:, c0:c0 + 512], pp[:],
                                                    recip[:, s:s + 1])
                    nc.sync.dma_start(
                        vatt_sb[(b0 + s) * G:(b0 + s + 1) * G, :], tmp[:])

                xqf_chunk(pair)

        if KVARIANT in ("pairs", "notrans"):
            nc.sync.dma_start(out[:, :DH], vatt_sb[:BPC, :])
            return
        # ---------------- tail: vaT transpose, fused fusion+classifier ----------
        with tc.tile_pool(name="tl", bufs=1) as tl:
            bf_sb = tl.tile([1, DH], F32R)
            nc.sync.dma_start(bf_sb[:], bf[:].bitcast(F32R))
            bc_sb = tl.tile([1, NANS], F32R)
            nc.sync.dma_start(bc_sb[:], bc[:].bitcast(F32R))

            # vaT[k]: [128, 64] bf16, columns g*16+b (own psum scope)
            vaT = []
            with tc.tile_pool(name="vat_ps", bufs=1, space="PSUM") as vat_ps:
                for k in range(NK):
                    t = tl.tile([128, G * BPC], BF16, tag=f"vaT{k}")
                    p = vat_ps.tile([128, G * BPC], F32, tag="pvat", bufs=2)
                    nc.tensor.transpose(p[:], vatt_sb[:, k * 128:(k + 1) * 128],
                                        ident[:G * BPC, :G * BPC])
                    for g in range(G):
                        nc.vector.tensor_copy(
                            t[:, g * BPC:(g + 1) * BPC],
                            p[:, g:G * BPC:G])
                    vaT.append(t)

            # staged: D (all glimpses) -> E (all k) -> classifier (j-outer)
            with tc.tile_pool(name="tl_ps", bufs=1, space="PSUM") as tl_ps:
                xv_sb = tl.tile([BPC, DH], F32)
                out_sb = tl.tile([BPC, NANS], F32)
                for g in range(G):
                    pd = tl_ps.tile([BPC, DHG], F32, tag="pd", bufs=2)
                    wts = []
                    for kk in range(NKM):
                        wt = wfp.tile([128, KM * DHG], BF16, tag="wfs", bufs=3)
                        nc.sync.dma_start(
                            wt[:].rearrange("p (k n) -> p k n", k=KM),
                            wf[g, kk * KM * 128:(kk + 1) * KM * 128, :]
                            .rearrange("(k c) n -> c k n", k=KM))
                        wts.append(wt)
                    for k in range(NK):
                        nc.tensor.matmul(
                            pd[:], vaT[k][:, g * BPC:(g + 1) * BPC],
                            wts[k // KM][:, (k % KM) * DHG:(k % KM + 1) * DHG],
                            start=(k == 0), stop=False, skip_group_check=True)
                    nc.tensor.matmul(pd[:], ones[:, :BPC],
                                     bf_sb[:, g * DHG:(g + 1) * DHG],
                                     start=False, stop=True,
                                     skip_group_check=True)
                    nc.scalar.activation(xv_sb[:, g * DHG:(g + 1) * DHG], pd[:],
                                         AF.Tanh)
                # E: x = tanh(xv * xqf) transposed into xT[k] tiles (bf16)
                xT = []
                for k in range(NK):
                    xmk = tl.tile([BPC, 128], F32, tag="xmk", bufs=3)
                    nc.vector.tensor_mul(xmk[:], xv_sb[:, k * 128:(k + 1) * 128],
                                         xqf_sb[:, k * 128:(k + 1) * 128])
                    px = tl_ps.tile([128, BPC], F32, tag="pxT", bufs=3)
                    nc.tensor.transpose(px[:], xmk[:], ident[:BPC, :BPC])
                    xTk = tl.tile([128, BPC], BF16, tag=f"xT{k}")
                    nc.scalar.activation(xTk[:], px[:], AF.Tanh)
                    xT.append(xTk)
                # classifier j-outer, Wc streamed 4-k-merged per (j, kk)
                for j, (n0, nw) in enumerate(NANS_TILES):
                    pc = tl_ps.tile([BPC, nw], F32, tag="pc", bufs=2)
                    for kk in range(NKM):
                        wct = tl.tile([128, KM * nw], BF16, tag="wcs", bufs=6)
                        nc.sync.dma_start(
                            wct[:].rearrange("p (k n) -> p k n", k=KM),
                            wc[kk * KM * 128:(kk + 1) * KM * 128, n0:n0 + nw]
                            .rearrange("(k c) n -> c k n", k=KM))
                        for ki in range(KM):
                            k = kk * KM + ki
                            nc.tensor.matmul(pc[:], xT[k][:],
                                             wct[:, ki * nw:(ki + 1) * nw],
                                             start=(k == 0), stop=False,
                                             skip_group_check=True)
                    nc.tensor.matmul(pc[:], ones[:, :BPC], bc_sb[:, n0:n0 + nw],
                                     start=False, stop=True,
                                     skip_group_check=True)
                    nc.vector.tensor_copy(out_sb[:, n0:n0 + nw], pc[:])
                nc.sync.dma_start(out[:], out_sb[:])


_module_cache = {}


def _get_module(reps: int = 1):
    if reps not in _module_cache:
        _module_cache[reps] = build_module(reps)
    return _module_cache[reps]


def make_in_maps(inputs: dict) -> list:
    iv = np.ascontiguousarray(inputs["input_v"], np.float32).reshape(B, DV, S)
    xq = np.ascontiguousarray(inputs["x_q_vec"], np.float32)
    shared = {
        "wv": np.ascontiguousarray(inputs["Wv_att"], np.float32),
        "bv": np.ascontiguousarray(inputs["bv_att"], np.float32).reshape(DA, 1),
        "wq": np.ascontiguousarray(inputs["Wq_att"], np.float32),
        "bq": np.ascontiguousarray(inputs["bq_att"], np.float32).reshape(DA, 1),
        "wa": np.ascontiguousarray(inputs["Wa"], np.float32),
        "ba": np.ascontiguousarray(inputs["ba"], np.float32).reshape(G, 1),
        "wf": np.ascontiguousarray(inputs["Wf"]).astype(ml_dtypes.bfloat16),
        "bf": np.ascontiguousarray(inputs["bf"], np.float32).reshape(1, DH),
        "wqf": np.ascontiguousarray(inputs["Wqf"], np.float32),
        "bqf": np.ascontiguousarray(inputs["bqf"], np.float32).reshape(1, DH),
        "wc": np.ascontiguousarray(inputs["Wc"]).astype(ml_dtypes.bfloat16),
        "bc": np.ascontiguousarray(inputs["bc"], np.float32).reshape(1, NANS),
    }
    in_maps = []
    for c in range(NCORES):
        m = dict(shared)
        m["v"] = np.ascontiguousarray(iv[c * BPC:(c + 1) * BPC])
        m["q"] = np.ascontiguousarray(xq[c * BPC:(c + 1) * BPC])
        in_maps.append(m)
    return in_maps


def kernel(**inputs) -> np.ndarray:
    nc = _get_module(1)
    in_maps = make_in_maps(inputs)
    res = run_bass_kernel_spmd(nc, in_maps, core_ids=list(range(NCORES)))
    return np.concatenate([res.results[c]["out"] for c in range(NCORES)], axis=0)



# revision 34
# speedup vs baseline: 1350.6271x; 1350.6271x over previous
"""Trainium2 Bass kernel for nn_AbstractAtt (MLB-style attention + fusion + classifier).

Data-parallel over 8 NeuronCores: batch 128 -> 16 samples/core, weights replicated.

V2: all matmul operands bf16 (PSUM accumulation f32; final rel err ~5e-3 vs the
2e-2 gate). All tensors are pre-packed on the host into DMA-friendly
[128, X]-contiguous blocks, including a pre-transposed v (vT) so the pooling
matmul needs no on-device PE transposes of v, and a pre-transposed q (qT).

Per-core pipeline:
  x_v   = tanh(v^T @ Wv + bv)            [DA, S2] per pair, PE + ACT(bias)
  x_att = tanh(x_v * x_q[b])             ACT scale, bf16 out
  scores= Wa^T @ x_att                   [G, S2] PSUM accumulation over DA tiles
  att   = exp(scores + ba); row sums via ACT accum_out; softmax denominator
          folded into the pooled output
  pool  = e^T @ vT                       vT streamed from host, [98, 2*DV] halves
  xv    = tanh(v_att @ Wf[g] + bf)       glimpse-packed vaT lhsT
  x     = tanh(xv * xqf);  out = x @ Wc + bc
"""

import os
from contextlib import ExitStack

import ml_dtypes
import numpy as np

import concourse.bass as bass
import concourse.mybir as mybir
import concourse.tile as tile
from concourse import bacc
from concourse.bass_utils import run_bass_kernel_spmd
from concourse.masks import make_identity

F32 = mybir.dt.float32
BF16 = mybir.dt.bfloat16
AF = mybir.ActivationFunctionType

# problem constants (hardcoded per contract)
B, DV, W, H = 128, 2048, 14, 14
S = W * H            # 196
DQ = 2048
DA = 1200
G = 4
DH = 2048
DHG = DH // G        # 512
NANS = 3000
NCORES = 8
BPC = B // NCORES    # 16 samples per core
NPAIR = BPC // 2     # 8 pairs

NK = DV // 128       # 16 k-tiles over DV (== DQ // 128)
DA_TILES = [(m * 128, min(128, DA - m * 128)) for m in range((DA + 127) // 128)]
NM = len(DA_TILES)   # 10
S2 = 2 * S           # 392 columns per pair
SH = S // 2          # 98 rows per vT half
NANS_TILES = [(j * 500, 500) for j in range(6)]
NJC = len(NANS_TILES)
XQF_TILES = [(j * 256, 256) for j in range(8)]


def build_module(reps: int = 1) -> bacc.Bacc:
    nc = bacc.Bacc("TRN2", target_bir_lowering=False, debug=False)

    t = {}
    t["qT"] = nc.dram_tensor("qT", [128, NK * BPC], BF16, kind="ExternalInput").ap()
    t["v"] = nc.dram_tensor("v", [NPAIR, 128, NK * S2], BF16, kind="ExternalInput").ap()
    t["vT"] = nc.dram_tensor("vT", [NPAIR, SH, 4 * DV], BF16, kind="ExternalInput").ap()
    t["wv"] = nc.dram_tensor("wv", [4, 128, 4 * DA], BF16, kind="ExternalInput").ap()
    t["wq"] = nc.dram_tensor("wq", [4, 128, 4 * DA], BF16, kind="ExternalInput").ap()
    t["wa"] = nc.dram_tensor("wa", [128, NM * G], BF16, kind="ExternalInput").ap()
    t["bv"] = nc.dram_tensor("bv", [128, NM], F32, kind="ExternalInput").ap()
    t["bq"] = nc.dram_tensor("bq", [128, NM], F32, kind="ExternalInput").ap()
    t["ba"] = nc.dram_tensor("ba", [G, 1], F32, kind="ExternalInput").ap()
    t["wqf"] = nc.dram_tensor("wqf", [8, 128, NK * 256], BF16, kind="ExternalInput").ap()
    t["bqf"] = nc.dram_tensor("bqf", [1, DH], BF16, kind="ExternalInput").ap()
    t["wf"] = nc.dram_tensor("wf", [G, 128, NK * DHG], BF16, kind="ExternalInput").ap()
    t["bf"] = nc.dram_tensor("bf", [1, DH], BF16, kind="ExternalInput").ap()
    t["wc"] = nc.dram_tensor("wc", [NJC, 128, NK * 500], BF16, kind="ExternalInput").ap()
    t["bc"] = nc.dram_tensor("bc", [1, NANS], BF16, kind="ExternalInput").ap()
    t["out"] = nc.dram_tensor("out", [BPC, NANS], F32, kind="ExternalOutput").ap()

    with tile.TileContext(nc) as tc:
        if reps > 4:
            with tc.For_i(0, reps, 1):
                emit_core(nc, tc, t)
        else:
            for _ in range(reps):
                emit_core(nc, tc, t)
    nc.compile()
    return nc


def emit_core(nc, tc, t):
    ctx = ExitStack()
    with ctx:
        # ---------------- persistent pools ----------------
        const_pool = ctx.enter_context(tc.tile_pool(name="const", bufs=1))

        ident = const_pool.tile([128, 128], F32)
        make_identity(nc, ident[:])
        ident_bf = const_pool.tile([128, 128], BF16)
        nc.vector.tensor_copy(ident_bf[:], ident[:])
        ones_bf = const_pool.tile([1, BPC], BF16)
        nc.gpsimd.memset(ones_bf[:], 1.0)

        # const DMAs are queued inside the startup weight stream (see below)
        qT = const_pool.tile([128, NK * BPC], BF16)
        bv_sb = const_pool.tile([128, NM], F32)
        wa_sb = const_pool.tile([128, NM * G], BF16)
        bq_sb = const_pool.tile([128, NM], F32)
        ba_sb = const_pool.tile([G, 1], F32)
        bqf_sb = const_pool.tile([1, DH], BF16)

        # xqT filled in pre-phase; xqf/vatt accumulate through the pair loop
        xqT = const_pool.tile([128, NM * BPC], F32)
        xqf_sb = const_pool.tile([BPC, DH], BF16)
        vatt_sb = const_pool.tile([G * BPC, DV], BF16)

        # Wf prefetch pool outlives the wv scope -> entered first (LIFO pools)
        wfp = ctx.enter_context(tc.tile_pool(name="wfp", bufs=1))

        # resident Wv (freed before tail): wv_sb[kk][:, ki*DA + d]
        wv_scope = tc.tile_pool(name="wvp", bufs=1)
        wv_pool = wv_scope.__enter__()
        wv_sb = []
        for kk in range(4):
            w = wv_pool.tile([128, 4 * DA], BF16, tag=f"wv{kk}")
            wv_sb.append(w)

        def wv_lhsT(k, m0, mw):
            return wv_sb[k // 4][:, (k % 4) * DA + m0:(k % 4) * DA + m0 + mw]

        # ---------------- pair loop (+ pre-phase overlapped with pair 0) ----
        # Startup ordering: pair-0's x_v matmuls only need wv + v0, so they
        # run FIRST while Wq streams in; the q-path pre-phase runs after, and
        # pair-0's attention (xa/scores on) is finished from saved xv tiles.
        wf_sb = {}
        with tc.tile_pool(name="pl", bufs=1) as pl, \
             tc.tile_pool(name="pl_ps", bufs=1, space="PSUM") as pl_ps:

            def xqf_chunk(j):
                n0, nw = XQF_TILES[j]
                wt = pl.tile([128, NK * 256], BF16, tag="wqf", bufs=1)
                nc.sync.dma_start(wt[:], t["wqf"][j])
                pj = pl_ps.tile([BPC, nw], F32, tag="peT", bufs=1)
                for k in range(NK):
                    nc.tensor.matmul(pj[:], qT[:, k * BPC:(k + 1) * BPC],
                                     wt[:, k * 256:(k + 1) * 256],
                                     start=(k == 0), stop=False,
                                     skip_group_check=True)
                nc.tensor.matmul(pj[:], ones_bf[:], bqf_sb[:, n0:n0 + nw],
                                 start=False, stop=True, skip_group_check=True)
                nc.scalar.activation(xqf_sb[:, n0:n0 + nw], pj[:], AF.Tanh)

            def attend(pair, p_sc, m, mw, xv_t):
                # xa = tanh(xv * xq[b]) both samples; accumulate scores
                b0 = pair * 2
                xa_t = pl.tile([128, S2], BF16, tag="xa", bufs=2)
                for s in range(2):
                    nc.scalar.activation(
                        xa_t[:mw, s * S:(s + 1) * S],
                        xv_t[:mw, s * S:(s + 1) * S], AF.Tanh,
                        scale=xqT[:mw, m * BPC + b0 + s:m * BPC + b0 + s + 1])
                nc.tensor.matmul(p_sc[:], wa_sb[:mw, m * G:(m + 1) * G],
                                 xa_t[:mw, :],
                                 start=(m == 0), stop=(m == NM - 1),
                                 skip_group_check=True)

            def finish_pair(pair, vT_sb, p_sc):
                b0 = pair * 2
                # att = exp(scores + ba), with per-sample row sums
                e_sb = pl.tile([G, S2], BF16, tag="e", bufs=1)
                esum = pl.tile([G, 2], F32, tag="esum", bufs=2)
                for s in range(2):
                    nc.scalar.activation(e_sb[:, s * S:(s + 1) * S],
                                         p_sc[:, s * S:(s + 1) * S], AF.Exp,
                                         bias=ba_sb[:], accum_out=esum[:, s:s + 1])
                recip = pl.tile([G, 2], F32, tag="recip", bufs=2)
                nc.vector.reciprocal(recip[:], esum[:])

                for s in range(2):
                    # e^T halves: [98, 2G] (col block h = spatial rows h*98..)
                    pt = pl_ps.tile([SH, 2 * G], BF16, tag="peT", bufs=1)
                    for h in range(2):
                        nc.tensor.transpose(
                            pt[:, h * G:(h + 1) * G],
                            e_sb[:, s * S + h * SH:s * S + (h + 1) * SH],
                            ident_bf[:G, :G])
                    eT = pl.tile([SH, 2 * G], BF16, tag="eT", bufs=2)
                    nc.vector.tensor_copy(eT[:], pt[:])

                    # pooling: U[g, c] = e^T @ v^T; normalize; 1 SBUF->SBUF DMA
                    tmp = pl.tile([G, DV], BF16, tag="ptmp", bufs=2)
                    for cc in range(DV // 512):
                        c0 = cc * 512
                        pp = pl_ps.tile([G, 512], F32, tag="ppool", bufs=3)
                        nc.tensor.matmul(pp[:], eT[:, 0:G],
                                         vT_sb[:, s * 2 * DV + c0:s * 2 * DV + c0 + 512],
                                         start=True, stop=False,
                                         skip_group_check=True)
                        nc.tensor.matmul(pp[:], eT[:, G:2 * G],
                                         vT_sb[:, (s * 2 + 1) * DV + c0:(s * 2 + 1) * DV + c0 + 512],
                                         start=False, stop=True,
                                         skip_group_check=True)
                        if cc < 2:
                            nc.vector.tensor_scalar_mul(
                                tmp[:, c0:c0 + 512], pp[:], recip[:, s:s + 1])
                        else:
                            nc.scalar.activation(
                                tmp[:, c0:c0 + 512], pp[:], AF.Copy,
                                scale=recip[:, s:s + 1])
                    # glimpse-major rows (g*BPC + b) via strided partition dst
                    nc.sync.dma_start(
                        vatt_sb[b0 + s::BPC, :], tmp[:])

            with tc.tile_pool(name="pre", bufs=1) as pre:
                # ---- startup DMA queue order (one serial channel):
                # wv0/v0 split so the first matmuls start (and warm the PE)
                # as early as possible; then wq/wv alternate; consts last
                v0_sb = pl.tile([128, NK * S2], BF16, tag="v", bufs=2)
                nc.sync.dma_start(wv_sb[0][:, :2 * DA], t["wv"][0][:, :2 * DA])
                nc.sync.dma_start(v0_sb[:, :2 * S2], t["v"][0][:, :2 * S2])
                nc.sync.dma_start(wv_sb[0][:, 2 * DA:], t["wv"][0][:, 2 * DA:])
                nc.sync.dma_start(v0_sb[:, 2 * S2:4 * S2],
                                  t["v"][0][:, 2 * S2:4 * S2])
                nc.sync.dma_start(qT[:], t["qT"][:])
                nc.sync.dma_start(bv_sb[:], t["bv"][:])
                nc.sync.dma_start(v0_sb[:, 4 * S2:8 * S2],
                                  t["v"][0][:, 4 * S2:8 * S2])
                nc.sync.dma_start(v0_sb[:, 8 * S2:], t["v"][0][:, 8 * S2:])
                wq_t = []
                for kk in range(4):
                    w = pre.tile([128, 4 * DA], BF16, tag="wq", bufs=2)
                    nc.sync.dma_start(w[:], t["wq"][kk])
                    wq_t.append(w)
                    if kk < 3:
                        nc.sync.dma_start(wv_sb[kk + 1][:], t["wv"][kk + 1])
                vT0_sb = pl.tile([SH, 4 * DV], BF16, tag="vt", bufs=1)
                nc.sync.dma_start(wa_sb[:], t["wa"][:])
                nc.sync.dma_start(bq_sb[:], t["bq"][:])
                nc.sync.dma_start(ba_sb[:], t["ba"][:])
                nc.sync.dma_start(bqf_sb[:], t["bqf"][:])

                # phases A+B in k-lockstep: pair-0 main matmuls (wv+v only)
                # interleaved with xq_lin k-groups, each gated on one wq part
                XL = [(0, 400, "ppool", 3), (400, 400, "ppool", 3),
                      (800, 400, "psc", 2)]
                pxq = [pl_ps.tile([BPC, nw], F32, tag=tg, bufs=bf,
                                  name=f"pxq{n0}")
                       for (n0, nw, tg, bf) in XL]
                xq_lin = pre.tile([BPC, DA], BF16)
                pm0 = pl_ps.tile([128, S2], F32, tag="pmain", bufs=2)

                def a_group(kk, pm, mw):
                    for ki in range(4):
                        k = kk * 4 + ki
                        nc.tensor.matmul(pm[:mw, :], wv_lhsT(k, 0, mw),
                                         v0_sb[:, k * S2:(k + 1) * S2],
                                         start=(k == 0), stop=(k == NK - 1),
                                         skip_group_check=True)

                def b_group(kk):
                    for ki in range(4):
                        k = kk * 4 + ki
                        for j, (n0, nw, tg, bf) in enumerate(XL):
                            nc.tensor.matmul(
                                pxq[j][:], qT[:, k * BPC:(k + 1) * BPC],
                                wq_t[kk][:, ki * DA + n0:ki * DA + n0 + nw],
                                start=(k == 0), stop=(k == NK - 1),
                                skip_group_check=True)

                for kk in range(4):
                    a_group(kk, pm0, DA_TILES[0][1])
                    if kk < 3:
                        b_group(kk)
                xv0 = []
                xv_t0 = pre.tile([128, S2], BF16, tag="xv0_0")
                nc.scalar.activation(xv_t0[:], pm0[:], AF.Tanh,
                                     bias=bv_sb[:, 0:1])
                xv0.append(xv_t0)
                # m=1 main tile covers the wq3 arrival window
                pm = pl_ps.tile([128, S2], F32, tag="pmain", bufs=2)
                a1_m0, a1_mw = DA_TILES[1]
                for k in range(NK):
                    nc.tensor.matmul(pm[:a1_mw, :], wv_lhsT(k, a1_m0, a1_mw),
                                     v0_sb[:, k * S2:(k + 1) * S2],
                                     start=(k == 0), stop=(k == NK - 1),
                                     skip_group_check=True)
                xv_t = pre.tile([128, S2], BF16, tag="xv0_1")
                nc.scalar.activation(xv_t[:a1_mw, :], pm[:a1_mw, :], AF.Tanh,
                                     bias=bv_sb[:a1_mw, 1:2])
                xv0.append(xv_t)
                b_group(3)
                for j, (n0, nw, tg, bf) in enumerate(XL):
                    nc.vector.tensor_copy(xq_lin[:, n0:n0 + nw], pxq[j][:])

                # remaining pair-0 main tiles
                for m in range(2, NM):
                    m0, mw = DA_TILES[m]
                    pm = pl_ps.tile([128, S2], F32, tag="pmain", bufs=2)
                    for k in range(NK):
                        nc.tensor.matmul(pm[:mw, :], wv_lhsT(k, m0, mw),
                                         v0_sb[:, k * S2:(k + 1) * S2],
                                         start=(k == 0), stop=(k == NK - 1),
                                         skip_group_check=True)
                    xv_t = pre.tile([128, S2], BF16, tag=f"xv0_{m}")
                    nc.scalar.activation(xv_t[:mw, :], pm[:mw, :], AF.Tanh,
                                         bias=bv_sb[:mw, m:m + 1])
                    xv0.append(xv_t)

                # x_qT[m] = tanh(xq_lin^T + bq) per DA tile  -> [mw, BPC] f32
                for m, (m0, mw) in enumerate(DA_TILES):
                    p = pl_ps.tile([128, BPC], BF16, tag="peT", bufs=1)
                    nc.tensor.transpose(p[:mw, :], xq_lin[:, m0:m0 + mw],
                                        ident_bf[:BPC, :BPC])
                    nc.scalar.activation(xqT[:mw, m * BPC:(m + 1) * BPC],
                                         p[:mw, :], AF.Tanh,
                                         bias=bq_sb[:mw, m:m + 1])

                # pair-0 attention interleaved into pair-1's main loop
                # (pair-0 xa/scores ride the ACT engine under pair-1 matmuls)
                p_sc0 = pl_ps.tile([G, S2], F32, tag="psc", bufs=2)
                v1_sb = pl.tile([128, NK * S2], BF16, tag="v", bufs=2)
                for c0, c1 in [(0, 2 * S2), (2 * S2, 4 * S2),
                               (4 * S2, 8 * S2), (8 * S2, NK * S2)]:
                    nc.sync.dma_start(v1_sb[:, c0:c1], t["v"][1][:, c0:c1])
                p_sc1 = pl_ps.tile([G, S2], F32, tag="psc", bufs=2)
                for m, (m0, mw) in enumerate(DA_TILES):
                    pm = pl_ps.tile([128, S2], F32, tag="pmain", bufs=2)
                    for k in range(NK):
                        nc.tensor.matmul(pm[:mw, :], wv_lhsT(k, m0, mw),
                                         v1_sb[:, k * S2:(k + 1) * S2],
                                         start=(k == 0), stop=(k == NK - 1),
                                         skip_group_check=True)
                    xv_t = pl.tile([128, S2], F32, tag="xv", bufs=2)
                    nc.scalar.activation(xv_t[:mw, :], pm[:mw, :], AF.Tanh,
                                         bias=bv_sb[:mw, m:m + 1])
                    attend(1, p_sc1, m, mw, xv_t)
                    attend(0, p_sc0, m, mw, xv0[m])
                nc.sync.dma_start(vT0_sb[:], t["vT"][0])
                finish_pair(0, vT0_sb, p_sc0)
                xqf_chunk(0)
                vT1_sb = pl.tile([SH, 4 * DV], BF16, tag="vt", bufs=1)
                nc.sync.dma_start(vT1_sb[:], t["vT"][1])
                finish_pair(1, vT1_sb, p_sc1)
            xqf_chunk(1)

            for pair in range(2, NPAIR):
                v_sb = pl.tile([128, NK * S2], BF16, tag="v", bufs=2)
                nc.sync.dma_start(v_sb[:], t["v"][pair])
                vT_sb = pl.tile([SH, 4 * DV], BF16, tag="vt", bufs=1)
                nc.sync.dma_start(vT_sb[:], t["vT"][pair])

                p_sc = pl_ps.tile([G, S2], F32, tag="psc", bufs=2)
                for m, (m0, mw) in enumerate(DA_TILES):
                    pm = pl_ps.tile([128, S2], F32, tag="pmain", bufs=2)
                    for k in range(NK):
                        nc.tensor.matmul(pm[:mw, :], wv_lhsT(k, m0, mw),
                                         v_sb[:, k * S2:(k + 1) * S2],
                                         start=(k == 0), stop=(k == NK - 1),
                                         skip_group_check=True)
                    xv_t = pl.tile([128, S2], F32, tag="xv", bufs=2)
                    nc.scalar.activation(xv_t[:mw, :], pm[:mw, :], AF.Tanh,
                                         bias=bv_sb[:mw, m:m + 1])
                    attend(pair, p_sc, m, mw, xv_t)
                finish_pair(pair, vT_sb, p_sc)

                xqf_chunk(pair)
                # prefetch all Wf + the first Wc chunk during the last pairs
                if pair >= NPAIR - 3:
                    g = pair - (NPAIR - 3)
                    w = wfp.tile([128, NK * DHG], BF16, tag=f"wf{g}")
                    nc.sync.dma_start(w[:], t["wf"][g])
                    wf_sb[g] = w
                if pair == NPAIR - 1:
                    wc0 = wfp.tile([128, NK * 500], BF16, tag="wc0")
                    nc.sync.dma_start(wc0[:], t["wc"][0])

        wv_scope.__exit__(None, None, None)

        # ---------------- tail: vaT transpose, fusion, classifier ----------
        with tc.tile_pool(name="tl", bufs=1) as tl, \
             tc.tile_pool(name="tl_ps", bufs=1, space="PSUM") as tl_ps:
            wf3 = tl.tile([128, NK * DHG], BF16, tag="wf3")
            nc.sync.dma_start(wf3[:], t["wf"][3])
            wf_sb[3] = wf3
            bf_sb = tl.tile([1, DH], BF16)
            nc.sync.dma_start(bf_sb[:], t["bf"][:])
            bc_sb = tl.tile([1, NANS], BF16)
            nc.sync.dma_start(bc_sb[:], t["bc"][:])
            wct = [wc0]
            for j in range(1, NJC):
                w = tl.tile([128, NK * 500], BF16, tag="wc", bufs=5)
                nc.sync.dma_start(w[:], t["wc"][j])
                wct.append(w)

            # vaT[k]: [128, G*BPC] bf16, columns already glimpse-major
            vaT = []
            for k in range(NK):
                tk = tl.tile([128, G * BPC], BF16, tag=f"vaT{k}")
                p = tl_ps.tile([128, G * BPC], BF16, tag="pvat", bufs=2)
                nc.tensor.transpose(p[:], vatt_sb[:, k * 128:(k + 1) * 128],
                                    ident_bf[:G * BPC, :G * BPC])
                nc.vector.tensor_copy(tk[:], p[:])
                vaT.append(tk)

            xv_sb = tl.tile([BPC, DH], F32)
            xT = []
            for g in range(G):
                pd = tl_ps.tile([BPC, DHG], F32, tag="pd", bufs=2)
                for k in range(NK):
                    nc.tensor.matmul(
                        pd[:], vaT[k][:, g * BPC:(g + 1) * BPC],
                        wf_sb[g][:, k * DHG:(k + 1) * DHG],
                        start=(k == 0), stop=False, skip_group_check=True)
                nc.tensor.matmul(pd[:], ones_bf[:],
                                 bf_sb[:, g * DHG:(g + 1) * DHG],
                                 start=False, stop=True, skip_group_check=True)
                nc.scalar.activation(xv_sb[:, g * DHG:(g + 1) * DHG], pd[:],
                                     AF.Tanh)
                # x = tanh(xv * xqf) transposed into xT[k] tiles for this g
                for k in range(g * (NK // G), (g + 1) * (NK // G)):
                    xmk = tl.tile([BPC, 128], F32, tag="xmk", bufs=3)
                    nc.vector.tensor_mul(xmk[:], xv_sb[:, k * 128:(k + 1) * 128],
                                         xqf_sb[:, k * 128:(k + 1) * 128])
                    px = tl_ps.tile([128, BPC], F32, tag="pxT", bufs=2)
                    nc.tensor.transpose(px[:], xmk[:], ident[:BPC, :BPC])
                    xTk = tl.tile([128, BPC], BF16, tag=f"xT{k}")
                    nc.scalar.activation(xTk[:], px[:], AF.Tanh)
                    xT.append(xTk)
            # classifier; per-chunk PSUM -> DRAM output DMA. The last
            # 500-wide chunk is split in half to shorten the end chain.
            CTILES = [(j, j * 500, 0, 500) for j in range(NJC - 1)]
            CTILES += [(NJC - 1, 2500, 0, 250), (NJC - 1, 2750, 250, 250)]
            for j, n0, w0, nw in CTILES:
                pc = tl_ps.tile([BPC, nw], F32, tag="pc", bufs=2)
                for k in range(NK):
                    nc.tensor.matmul(pc[:], xT[k][:],
                                     wct[j][:, k * 500 + w0:k * 500 + w0 + nw],
                                     start=(k == 0), stop=False,
                                     skip_group_check=True)
                nc.tensor.matmul(pc[:], ones_bf[:], bc_sb[:, n0:n0 + nw],
                                 start=False, stop=True, skip_group_check=True)
                oj = tl.tile([BPC, nw], F32, tag="oj", bufs=7)
                nc.vector.tensor_copy(oj[:], pc[:])
                nc.sync.dma_start(t["out"][:, n0:n0 + nw], oj[:])


_module_cache = {}


def _get_module(reps: int = 1):
    if reps not in _module_cache:
        _module_cache[reps] = build_module(reps)
    return _module_cache[reps]


def make_in_maps(inputs: dict) -> list:
    bf = ml_dtypes.bfloat16
    iv = np.ascontiguousarray(inputs["input_v"], np.float32).reshape(B, DV, S)
    xq = np.ascontiguousarray(inputs["x_q_vec"], np.float32)

    def kpack(w, ncol):
        # [DQ, ncol] -> [4, 128, 4*ncol] (k-major groups of 4)
        w = np.asarray(w, np.float32).astype(bf)
        return np.ascontiguousarray(
            w.reshape(4, 4, 128, ncol).transpose(0, 2, 1, 3).reshape(4, 128, 4 * ncol))

    wa = np.zeros((NM * 128, G), np.float32)
    wa[:DA] = np.asarray(inputs["Wa"], np.float32)
    bvp = np.zeros((NM * 128,), np.float32)
    bvp[:DA] = np.asarray(inputs["bv_att"], np.float32)
    bqp = np.zeros((NM * 128,), np.float32)
    bqp[:DA] = np.asarray(inputs["bq_att"], np.float32)

    wqf = np.asarray(inputs["Wqf"], np.float32).astype(bf)
    wf = np.asarray(inputs["Wf"], np.float32).astype(bf)
    wc = np.asarray(inputs["Wc"], np.float32).astype(bf)

    shared = {
        "wv": kpack(inputs["Wv_att"], DA),
        "wq": kpack(inputs["Wq_att"], DA),
        "wa": np.ascontiguousarray(
            wa.astype(bf).reshape(NM, 128, G).transpose(1, 0, 2).reshape(128, NM * G)),
        "bv": np.ascontiguousarray(bvp.reshape(NM, 128).T),
        "bq": np.ascontiguousarray(bqp.reshape(NM, 128).T),
        "ba": np.ascontiguousarray(inputs["ba"], np.float32).reshape(G, 1),
        "wqf": np.ascontiguousarray(
            wqf.reshape(16, 128, 8, 256).transpose(2, 1, 0, 3).reshape(8, 128, NK * 256)),
        "bqf": np.ascontiguousarray(inputs["bqf"], np.float32).astype(bf).reshape(1, DH),
        "wf": np.ascontiguousarray(
            wf.reshape(G, 16, 128, DHG).transpose(0, 2, 1, 3).reshape(G, 128, NK * DHG)),
        "bf": np.ascontiguousarray(inputs["bf"], np.float32).astype(bf).reshape(1, DH),
        "wc": np.ascontiguousarray(
            wc.reshape(16, 128, NJC, 500).transpose(2, 1, 0, 3).reshape(NJC, 128, NK * 500)),
        "bc": np.ascontiguousarray(inputs["bc"], np.float32).astype(bf).reshape(1, NANS),
    }
    in_maps = []
    for c in range(NCORES):
        vs = iv[c * BPC:(c + 1) * BPC].astype(bf)          # [16, DV, S]
        qs = xq[c * BPC:(c + 1) * BPC].astype(bf)          # [16, DQ]
        m = dict(shared)
        m["qT"] = np.ascontiguousarray(
            qs.T.reshape(NK, 128, BPC).transpose(1, 0, 2).reshape(128, NK * BPC))
        m["v"] = np.ascontiguousarray(
            vs.reshape(NPAIR, 2, NK, 128, S).transpose(0, 3, 2, 1, 4)
            .reshape(NPAIR, 128, NK * S2))
        m["vT"] = np.ascontiguousarray(
            vs.transpose(0, 2, 1).reshape(NPAIR, 2, 2, SH, DV)
            .transpose(0, 3, 1, 2, 4).reshape(NPAIR, SH, 4 * DV))
        in_maps.append(m)
    return in_maps


def kernel(**inputs) -> np.ndarray:
    nc = _get_module(1)
    in_maps = make_in_maps(inputs)
    res = run_bass_kernel_spmd(nc, in_maps, core_ids=list(range(NCORES)))
    return np.concatenate([res.results[c]["out"] for c in range(NCORES)], axis=0)


# revision 36
# speedup vs baseline: 1992.5890x; 1.4753x over previous
"""Trainium2 Bass kernel for nn_AbstractAtt (MLB-style attention + fusion + classifier).

Data-parallel over 8 NeuronCores: batch 128 -> 16 samples/core, weights replicated.

V2: all matmul operands bf16 (PSUM accumulation f32; final rel err ~5e-3 vs the
2e-2 gate). All tensors are pre-packed on the host into DMA-friendly
[128, X]-contiguous blocks, including a pre-transposed v (vT) so the pooling
matmul needs no on-device PE transposes of v, and a pre-transposed q (qT).

Per-core pipeline:
  x_v   = tanh(v^T @ Wv + bv)            [DA, S2] per pair, PE + ACT(bias)
  x_att = tanh(x_v * x_q[b])             ACT scale, bf16 out
  scores= Wa^T @ x_att                   [G, S2] PSUM accumulation over DA tiles
  att   = exp(scores + ba); row sums via ACT accum_out; softmax denominator
          folded into the pooled output
  pool  = e^T @ vT                       vT streamed from host, [98, 2*DV] halves
  xv    = tanh(v_att @ Wf[g] + bf)       glimpse-packed vaT lhsT
  x     = tanh(xv * xqf);  out = x @ Wc + bc
"""

from contextlib import ExitStack

import ml_dtypes
import numpy as np

import concourse.mybir as mybir
import concourse.tile as tile
from concourse import bacc
from concourse.bass_utils import run_bass_kernel_spmd
from concourse.masks import make_identity

F32 = mybir.dt.float32
BF16 = mybir.dt.bfloat16
AF = mybir.ActivationFunctionType

# problem constants (hardcoded per contract)
B, DV, W, H = 128, 2048, 14, 14
S = W * H            # 196
DQ = 2048
DA = 1200
G = 4
DH = 2048
DHG = DH // G        # 512
NANS = 3000
NCORES = 8
BPC = B // NCORES    # 16 samples per core
NPAIR = BPC // 2     # 8 pairs

NK = DV // 128       # 16 k-tiles over DV (== DQ // 128)
DA_TILES = [(m * 128, min(128, DA - m * 128)) for m in range((DA + 127) // 128)]
NM = len(DA_TILES)   # 10
S2 = 2 * S           # 392 columns per pair
SH = S // 2          # 98 rows per vT half
NANS_TILES = [(j * 500, 500) for j in range(6)]
NJC = len(NANS_TILES)
XQF_TILES = [(j * 256, 256) for j in range(8)]


def build_module(reps: int = 1) -> bacc.Bacc:
    nc = bacc.Bacc("TRN2", target_bir_lowering=False, debug=False)

    t = {}
    t["qT"] = nc.dram_tensor("qT", [128, NK * BPC], BF16, kind="ExternalInput").ap()
    t["v"] = nc.dram_tensor("v", [NPAIR, 128, NK * S2], BF16, kind="ExternalInput").ap()
    t["vT"] = nc.dram_tensor("vT", [NPAIR, SH, 4 * DV], BF16, kind="ExternalInput").ap()
    t["wv"] = nc.dram_tensor("wv", [4, 128, 4 * DA], BF16, kind="ExternalInput").ap()
    t["wq"] = nc.dram_tensor("wq", [4, 128, 4 * DA], BF16, kind="ExternalInput").ap()
    t["wa"] = nc.dram_tensor("wa", [128, NM * G], BF16, kind="ExternalInput").ap()
    t["bv"] = nc.dram_tensor("bv", [128, NM], F32, kind="ExternalInput").ap()
    t["bq"] = nc.dram_tensor("bq", [128, NM], F32, kind="ExternalInput").ap()
    t["ba"] = nc.dram_tensor("ba", [G, 1], F32, kind="ExternalInput").ap()
    t["wqf"] = nc.dram_tensor("wqf", [8, 128, NK * 256], BF16, kind="ExternalInput").ap()
    t["bqf"] = nc.dram_tensor("bqf", [1, DH], BF16, kind="ExternalInput").ap()
    t["wf"] = nc.dram_tensor("wf", [G, 128, NK * DHG], BF16, kind="ExternalInput").ap()
    t["bf"] = nc.dram_tensor("bf", [1, DH], BF16, kind="ExternalInput").ap()
    t["wc"] = nc.dram_tensor("wc", [NJC, 128, NK * 500], BF16, kind="ExternalInput").ap()
    t["bc"] = nc.dram_tensor("bc", [1, NANS], BF16, kind="ExternalInput").ap()
    t["out"] = nc.dram_tensor("out", [BPC, NANS], F32, kind="ExternalOutput").ap()

    with tile.TileContext(nc) as tc:
        if reps > 4:
            with tc.For_i(0, reps, 1):
                emit_core(nc, tc, t)
        else:
            for _ in range(reps):
                emit_core(nc, tc, t)
    nc.compile()
    return nc


def emit_core(nc, tc, t):
    ctx = ExitStack()
    with ctx:
        # ---------------- persistent pools ----------------
        const_pool = ctx.enter_context(tc.tile_pool(name="const", bufs=1))

        ident = const_pool.tile([128, 128], F32)
        make_identity(nc, ident[:])
        ident_bf = const_pool.tile([128, 128], BF16)
        nc.vector.tensor_copy(ident_bf[:], ident[:])
        ones_bf = const_pool.tile([1, BPC], BF16)
        nc.gpsimd.memset(ones_bf[:], 1.0)

        # const DMAs are queued inside the startup weight stream (see below)
        qT = const_pool.tile([128, NK * BPC], BF16)
        bv_sb = const_pool.tile([128, NM], F32)
        wa_sb = const_pool.tile([128, NM * G], BF16)
        bq_sb = const_pool.tile([128, NM], F32)
        ba_sb = const_pool.tile([G, 1], F32)
        bqf_sb = const_pool.tile([1, DH], BF16)

        # xqT filled in pre-phase; xqf/vatt accumulate through the pair loop
        xqT = const_pool.tile([128, NM * BPC], F32)
        xqf_sb = const_pool.tile([BPC, DH], BF16)
        vatt_sb = const_pool.tile([G * BPC, DV], BF16)

        # Wf prefetch pool outlives the wv scope -> entered first (LIFO pools)
        wfp = ctx.enter_context(tc.tile_pool(name="wfp", bufs=1))

        # resident Wv (freed before tail): wv_sb[kk][:, ki*DA + d]
        wv_scope = tc.tile_pool(name="wvp", bufs=1)
        wv_pool = wv_scope.__enter__()
        wv_sb = []
        for kk in range(4):
            w = wv_pool.tile([128, 4 * DA], BF16, tag=f"wv{kk}")
            wv_sb.append(w)

        def wv_lhsT(k, m0, mw):
            return wv_sb[k // 4][:, (k % 4) * DA + m0:(k % 4) * DA + m0 + mw]

        # ---------------- pair loop (+ pre-phase overlapped with pair 0) ----
        # Startup ordering: pair-0's x_v matmuls only need wv + v0, so they
        # run FIRST while Wq streams in; the q-path pre-phase runs after, and
        # pair-0's attention (xa/scores on) is finished from saved xv tiles.
        wf_sb = {}
        with tc.tile_pool(name="pl", bufs=1) as pl, \
             tc.tile_pool(name="pl_ps", bufs=1, space="PSUM") as pl_ps:

            def xqf_chunk(j):
                n0, nw = XQF_TILES[j]
                wt = pl.tile([128, NK * 256], BF16, tag="wqf", bufs=1)
                nc.sync.dma_start(wt[:], t["wqf"][j])
                pj = pl_ps.tile([BPC, nw], F32, tag="peT", bufs=1)
                for k in range(NK):
                    nc.tensor.matmul(pj[:], qT[:, k * BPC:(k + 1) * BPC],
                                     wt[:, k * 256:(k + 1) * 256],
                                     start=(k == 0), stop=False,
                                     skip_group_check=True)
                nc.tensor.matmul(pj[:], ones_bf[:], bqf_sb[:, n0:n0 + nw],
                                 start=False, stop=True, skip_group_check=True)
                nc.scalar.activation(xqf_sb[:, n0:n0 + nw], pj[:], AF.Tanh)

            def attend(pair, p_sc, m, mw, xv_t):
                # xa = tanh(xv * xq[b]) both samples; accumulate scores
                b0 = pair * 2
                xa_t = pl.tile([128, S2], BF16, tag="xa", bufs=2)
                for s in range(2):
                    nc.scalar.activation(
                        xa_t[:mw, s * S:(s + 1) * S],
                        xv_t[:mw, s * S:(s + 1) * S], AF.Tanh,
                        scale=xqT[:mw, m * BPC + b0 + s:m * BPC + b0 + s + 1])
                nc.tensor.matmul(p_sc[:], wa_sb[:mw, m * G:(m + 1) * G],
                                 xa_t[:mw, :],
                                 start=(m == 0), stop=(m == NM - 1),
                                 skip_group_check=True)

            def finish_pair(pair, vT_sb, p_sc):
                b0 = pair * 2
                # att = exp(scores + ba), with per-sample row sums
                e_sb = pl.tile([G, S2], BF16, tag="e", bufs=1)
                esum = pl.tile([G, 2], F32, tag="esum", bufs=2)
                for s in range(2):
                    nc.scalar.activation(e_sb[:, s * S:(s + 1) * S],
                                         p_sc[:, s * S:(s + 1) * S], AF.Exp,
                                         bias=ba_sb[:], accum_out=esum[:, s:s + 1])
                recip = pl.tile([G, 2], F32, tag="recip", bufs=2)
                nc.vector.reciprocal(recip[:], esum[:])

                for s in range(2):
                    # e^T halves: [98, 2G] (col block h = spatial rows h*98..)
                    pt = pl_ps.tile([SH, 2 * G], BF16, tag="peT", bufs=1)
                    for h in range(2):
                        nc.tensor.transpose(
                            pt[:, h * G:(h + 1) * G],
                            e_sb[:, s * S + h * SH:s * S + (h + 1) * SH],
                            ident_bf[:G, :G])
                    eT = pl.tile([SH, 2 * G], BF16, tag="eT", bufs=2)
                    nc.vector.tensor_copy(eT[:], pt[:])

                    # pooling: U[g, c] = e^T @ v^T; normalize; 1 SBUF->SBUF DMA
                    tmp = pl.tile([G, DV], BF16, tag="ptmp", bufs=2)
                    for cc in range(DV // 512):
                        c0 = cc * 512
                        pp = pl_ps.tile([G, 512], F32, tag="ppool", bufs=3)
                        nc.tensor.matmul(pp[:], eT[:, 0:G],
                                         vT_sb[:, s * 2 * DV + c0:s * 2 * DV + c0 + 512],
                                         start=True, stop=False,
                                         skip_group_check=True)
                        nc.tensor.matmul(pp[:], eT[:, G:2 * G],
                                         vT_sb[:, (s * 2 + 1) * DV + c0:(s * 2 + 1) * DV + c0 + 512],
                                         start=False, stop=True,
                                         skip_group_check=True)
                        if cc < 2:
                            nc.vector.tensor_scalar_mul(
                                tmp[:, c0:c0 + 512], pp[:], recip[:, s:s + 1])
                        else:
                            nc.scalar.activation(
                                tmp[:, c0:c0 + 512], pp[:], AF.Copy,
                                scale=recip[:, s:s + 1])
                    # glimpse-major rows (g*BPC + b) via strided partition dst
                    nc.sync.dma_start(
                        vatt_sb[b0 + s::BPC, :], tmp[:])

            with tc.tile_pool(name="pre", bufs=1) as pre:
                XL = [(0, 400, "ppool", 3), (400, 400, "ppool", 3),
                      (800, 400, "psc", 2)]
                # ---- startup DMA queue order (one serial channel):
                # wv0/v0 split so the first matmuls start (and warm the PE)
                # as early as possible; then wq/wv alternate; consts last
                v0_sb = pl.tile([128, NK * S2], BF16, tag="v", bufs=2)
                nc.sync.dma_start(wv_sb[0][:, :2 * DA], t["wv"][0][:, :2 * DA])
                nc.sync.dma_start(v0_sb[:, :2 * S2], t["v"][0][:, :2 * S2])
                nc.sync.dma_start(wv_sb[0][:, 2 * DA:], t["wv"][0][:, 2 * DA:])
                nc.sync.dma_start(v0_sb[:, 2 * S2:4 * S2],
                                  t["v"][0][:, 2 * S2:4 * S2])
                nc.sync.dma_start(qT[:], t["qT"][:])
                nc.sync.dma_start(bv_sb[:], t["bv"][:])
                nc.sync.dma_start(v0_sb[:, 4 * S2:8 * S2],
                                  t["v"][0][:, 4 * S2:8 * S2])
                nc.sync.dma_start(v0_sb[:, 8 * S2:], t["v"][0][:, 8 * S2:])
                wq_t = []
                for kk in range(4):
                    w = pre.tile([128, 4 * DA], BF16, tag="wq", bufs=2)
                    wv_ = w[:].rearrange("p (k n) -> p k n", k=4)
                    dv_ = t["wq"][kk].rearrange("p (k n) -> p k n", k=4)
                    for (n0, nw, tg, bf) in XL:
                        nc.sync.dma_start(wv_[:, :, n0:n0 + nw],
                                          dv_[:, :, n0:n0 + nw])
                    wq_t.append(w)
                    if kk < 3:
                        nc.sync.dma_start(wv_sb[kk + 1][:], t["wv"][kk + 1])
                vT0_sb = pl.tile([SH, 4 * DV], BF16, tag="vt", bufs=1)
                nc.sync.dma_start(wa_sb[:], t["wa"][:])
                nc.sync.dma_start(bq_sb[:], t["bq"][:])
                nc.sync.dma_start(ba_sb[:], t["ba"][:])
                nc.sync.dma_start(bqf_sb[:], t["bqf"][:])

                # phases A+B in k-lockstep: pair-0 main matmuls (wv+v only)
                # interleaved with xq_lin k-groups, each gated on one wq slice
                pxq = [pl_ps.tile([BPC, nw], F32, tag=tg, bufs=bf,
                                  name=f"pxq{n0}")
                       for (n0, nw, tg, bf) in XL]
                xq_lin = pre.tile([BPC, DA], BF16)
                pm0 = pl_ps.tile([128, S2], F32, tag="pmain", bufs=2)

                def a_group(kk, pm, mw):
                    for ki in range(4):
                        k = kk * 4 + ki
                        nc.tensor.matmul(pm[:mw, :], wv_lhsT(k, 0, mw),
                                         v0_sb[:, k * S2:(k + 1) * S2],
                                         start=(k == 0), stop=(k == NK - 1),
                                         skip_group_check=True)

                def b_group(kk):
                    for j, (n0, nw, tg, bf) in enumerate(XL):
                        for ki in range(4):
                            k = kk * 4 + ki
                            nc.tensor.matmul(
                                pxq[j][:], qT[:, k * BPC:(k + 1) * BPC],
                                wq_t[kk][:, ki * DA + n0:ki * DA + n0 + nw],
                                start=(k == 0), stop=(k == NK - 1),
                                skip_group_check=True)

                for kk in range(4):
                    a_group(kk, pm0, DA_TILES[0][1])
                    if kk < 3:
                        b_group(kk)
                xv0 = []
                xv_t0 = pre.tile([128, S2], BF16, tag="xv0_0")
                nc.scalar.activation(xv_t0[:], pm0[:], AF.Tanh,
                                     bias=bv_sb[:, 0:1])
                xv0.append(xv_t0)
                # m=1 main tile covers the wq3 arrival window
                pm = pl_ps.tile([128, S2], F32, tag="pmain", bufs=2)
                a1_m0, a1_mw = DA_TILES[1]
                for k in range(NK):
                    nc.tensor.matmul(pm[:a1_mw, :], wv_lhsT(k, a1_m0, a1_mw),
                                     v0_sb[:, k * S2:(k + 1) * S2],
                                     start=(k == 0), stop=(k == NK - 1),
                                     skip_group_check=True)
                xv_t = pre.tile([128, S2], BF16, tag="xv0_1")
                nc.scalar.activation(xv_t[:a1_mw, :], pm[:a1_mw, :], AF.Tanh,
                                     bias=bv_sb[:a1_mw, 1:2])
                xv0.append(xv_t)
                b_group(3)
                for j, (n0, nw, tg, bf) in enumerate(XL):
                    nc.vector.tensor_copy(xq_lin[:, n0:n0 + nw], pxq[j][:])

                # remaining pair-0 main tiles
                for m in range(2, NM):
                    m0, mw = DA_TILES[m]
                    pm = pl_ps.tile([128, S2], F32, tag="pmain", bufs=2)
                    for k in range(NK):
                        nc.tensor.matmul(pm[:mw, :], wv_lhsT(k, m0, mw),
                                         v0_sb[:, k * S2:(k + 1) * S2],
                                         start=(k == 0), stop=(k == NK - 1),
                                         skip_group_check=True)
                    xv_t = pre.tile([128, S2], BF16, tag=f"xv0_{m}")
                    nc.scalar.activation(xv_t[:mw, :], pm[:mw, :], AF.Tanh,
                                         bias=bv_sb[:mw, m:m + 1])
                    xv0.append(xv_t)

                # x_qT[m] = tanh(xq_lin^T + bq) per DA tile  -> [mw, BPC] f32
                for m, (m0, mw) in enumerate(DA_TILES):
                    p = pl_ps.tile([128, BPC], BF16, tag="peT", bufs=1)
                    nc.tensor.transpose(p[:mw, :], xq_lin[:, m0:m0 + mw],
                                        ident_bf[:BPC, :BPC])
                    nc.scalar.activation(xqT[:mw, m * BPC:(m + 1) * BPC],
                                         p[:mw, :], AF.Tanh,
                                         bias=bq_sb[:mw, m:m + 1])

                # pair-0 attention interleaved into pair-1's main loop
                # (pair-0 xa/scores ride the ACT engine under pair-1 matmuls)
                p_sc0 = pl_ps.tile([G, S2], F32, tag="psc", bufs=2)
                v1_sb = pl.tile([128, NK * S2], BF16, tag="v", bufs=2)
                for c0, c1 in [(0, 2 * S2), (2 * S2, 4 * S2),
                               (4 * S2, 8 * S2), (8 * S2, NK * S2)]:
                    nc.sync.dma_start(v1_sb[:, c0:c1], t["v"][1][:, c0:c1])
                p_sc1 = pl_ps.tile([G, S2], F32, tag="psc", bufs=2)
                for m, (m0, mw) in enumerate(DA_TILES):
                    pm = pl_ps.tile([128, S2], F32, tag="pmain", bufs=2)
                    for k in range(NK):
                        nc.tensor.matmul(pm[:mw, :], wv_lhsT(k, m0, mw),
                                         v1_sb[:, k * S2:(k + 1) * S2],
                                         start=(k == 0), stop=(k == NK - 1),
                                         skip_group_check=True)
                    xv_t = pl.tile([128, S2], F32, tag="xv", bufs=2)
                    nc.scalar.activation(xv_t[:mw, :], pm[:mw, :], AF.Tanh,
                                         bias=bv_sb[:mw, m:m + 1])
                    attend(1, p_sc1, m, mw, xv_t)
                    attend(0, p_sc0, m, mw, xv0[m])
                nc.sync.dma_start(vT0_sb[:], t["vT"][0])
                finish_pair(0, vT0_sb, p_sc0)
                xqf_chunk(0)
                vT1_sb = pl.tile([SH, 4 * DV], BF16, tag="vt", bufs=1)
                nc.sync.dma_start(vT1_sb[:], t["vT"][1])
                finish_pair(1, vT1_sb, p_sc1)
            xqf_chunk(1)

            for pair in range(2, NPAIR):
                v_sb = pl.tile([128, NK * S2], BF16, tag="v", bufs=2)
                nc.sync.dma_start(v_sb[:], t["v"][pair])
                vT_sb = pl.tile([SH, 4 * DV], BF16, tag="vt", bufs=1)
                nc.sync.dma_start(vT_sb[:], t["vT"][pair])

                p_sc = pl_ps.tile([G, S2], F32, tag="psc", bufs=2)
                for m, (m0, mw) in enumerate(DA_TILES):
                    pm = pl_ps.tile([128, S2], F32, tag="pmain", bufs=2)
                    for k in range(NK):
                        nc.tensor.matmul(pm[:mw, :], wv_lhsT(k, m0, mw),
                                         v_sb[:, k * S2:(k + 1) * S2],
                                         start=(k == 0), stop=(k == NK - 1),
                                         skip_group_check=True)
                    xv_t = pl.tile([128, S2], F32, tag="xv", bufs=2)
                    nc.scalar.activation(xv_t[:mw, :], pm[:mw, :], AF.Tanh,
                                         bias=bv_sb[:mw, m:m + 1])
                    attend(pair, p_sc, m, mw, xv_t)
                finish_pair(pair, vT_sb, p_sc)

                xqf_chunk(pair)
                # prefetch all Wf + the first Wc chunk during the last pairs
                if pair >= NPAIR - 3:
                    g = pair - (NPAIR - 3)
                    w = wfp.tile([128, NK * DHG], BF16, tag=f"wf{g}")
                    nc.sync.dma_start(w[:], t["wf"][g])
                    wf_sb[g] = w
                if pair == NPAIR - 1:
                    wc0 = wfp.tile([128, NK * 500], BF16, tag="wc0")
                    nc.sync.dma_start(wc0[:], t["wc"][0])

        wv_scope.__exit__(None, None, None)

        # ---------------- tail: vaT transpose, fusion, classifier ----------
        with tc.tile_pool(name="tl", bufs=1) as tl, \
             tc.tile_pool(name="tl_ps", bufs=1, space="PSUM") as tl_ps:
            wf3 = tl.tile([128, NK * DHG], BF16, tag="wf3")
            nc.sync.dma_start(wf3[:], t["wf"][3])
            wf_sb[3] = wf3
            bf_sb = tl.tile([1, DH], BF16)
            nc.sync.dma_start(bf_sb[:], t["bf"][:])
            bc_sb = tl.tile([1, NANS], BF16)
            nc.sync.dma_start(bc_sb[:], t["bc"][:])
            wct = [wc0]
            for j in range(1, NJC):
                w = tl.tile([128, NK * 500], BF16, tag="wc", bufs=5)
                nc.sync.dma_start(w[:], t["wc"][j])
                wct.append(w)

            # vaT[k]: [128, G*BPC] bf16, columns already glimpse-major
            vaT = []
            for k in range(NK):
                tk = tl.tile([128, G * BPC], BF16, tag=f"vaT{k}")
                p = tl_ps.tile([128, G * BPC], BF16, tag="pvat", bufs=2)
                nc.tensor.transpose(p[:], vatt_sb[:, k * 128:(k + 1) * 128],
                                    ident_bf[:G * BPC, :G * BPC])
                nc.vector.tensor_copy(tk[:], p[:])
                vaT.append(tk)

            xv_sb = tl.tile([BPC, DH], F32)
            xT = []
            for g in range(G):
                pd = tl_ps.tile([BPC, DHG], F32, tag="pd", bufs=2)
                for k in range(NK):
                    nc.tensor.matmul(
                        pd[:], vaT[k][:, g * BPC:(g + 1) * BPC],
                        wf_sb[g][:, k * DHG:(k + 1) * DHG],
                        start=(k == 0), stop=False, skip_group_check=True)
                nc.tensor.matmul(pd[:], ones_bf[:],
                                 bf_sb[:, g * DHG:(g + 1) * DHG],
                                 start=False, stop=True, skip_group_check=True)
                nc.scalar.activation(xv_sb[:, g * DHG:(g + 1) * DHG], pd[:],
                                     AF.Tanh)
                # x = tanh(xv * xqf) transposed into xT[k] tiles for this g
                for k in range(g * (NK // G), (g + 1) * (NK // G)):
                    xmk = tl.tile([BPC, 128], F32, tag="xmk", bufs=3)
                    nc.vector.tensor_mul(xmk[:], xv_sb[:, k * 128:(k + 1) * 128],
                                         xqf_sb[:, k * 128:(k + 1) * 128])
                    px = tl_ps.tile([128, BPC], F32, tag="pxT", bufs=2)
                    nc.tensor.transpose(px[:], xmk[:], ident[:BPC, :BPC])
                    xTk = tl.tile([128, BPC], BF16, tag=f"xT{k}")
                    nc.scalar.activation(xTk[:], px[:], AF.Tanh)
                    xT.append(xTk)
            # classifier; per-chunk PSUM -> DRAM output DMA. The last
            # 500-wide chunk is split in half to shorten the end chain.
            CTILES = [(j, j * 500, 0, 500) for j in range(NJC - 1)]
            CTILES += [(NJC - 1, 2500, 0, 250), (NJC - 1, 2750, 250, 250)]
            for j, n0, w0, nw in CTILES:
                pc = tl_ps.tile([BPC, nw], F32, tag="pc", bufs=2)
                for k in range(NK):
                    nc.tensor.matmul(pc[:], xT[k][:],
                                     wct[j][:, k * 500 + w0:k * 500 + w0 + nw],
                                     start=(k == 0), stop=False,
                                     skip_group_check=True)
                nc.tensor.matmul(pc[:], ones_bf[:], bc_sb[:, n0:n0 + nw],
                                 start=False, stop=True, skip_group_check=True)
                oj = tl.tile([BPC, nw], F32, tag="oj", bufs=7)
                nc.vector.tensor_copy(oj[:], pc[:])
                nc.sync.dma_start(t["out"][:, n0:n0 + nw], oj[:])


_module_cache = {}


def _get_module(reps: int = 1):
    if reps not in _module_cache:
        _module_cache[reps] = build_module(reps)
    return _module_cache[reps]


def make_in_maps(inputs: dict) -> list:
    bf = ml_dtypes.bfloat16
    iv = np.ascontiguousarray(inputs["input_v"], np.float32).reshape(B, DV, S)
    xq = np.ascontiguousarray(inputs["x_q_vec"], np.float32)

    def kpack(w, ncol):
        # [DQ, ncol] -> [4, 128, 4*ncol] (k-major groups of 4)
        w = np.asarray(w, np.float32).astype(bf)
        return np.ascontiguousarray(
            w.reshape(4, 4, 128, ncol).transpose(0, 2, 1, 3).reshape(4, 128, 4 * ncol))

    wa = np.zeros((NM * 128, G), np.float32)
    wa[:DA] = np.asarray(inputs["Wa"], np.float32)
    bvp = np.zeros((NM * 128,), np.float32)
    bvp[:DA] = np.asarray(inputs["bv_att"], np.float32)
    bqp = np.zeros((NM * 128,), np.float32)
    bqp[:DA] = np.asarray(inputs["bq_att"], np.float32)

    wqf = np.asarray(inputs["Wqf"], np.float32).astype(bf)
    wf = np.asarray(inputs["Wf"], np.float32).astype(bf)
    wc = np.asarray(inputs["Wc"], np.float32).astype(bf)

    shared = {
        "wv": kpack(inputs["Wv_att"], DA),
        "wq": kpack(inputs["Wq_att"], DA),
        "wa": np.ascontiguousarray(
            wa.astype(bf).reshape(NM, 128, G).transpose(1, 0, 2).reshape(128, NM * G)),
        "bv": np.ascontiguousarray(bvp.reshape(NM, 128).T),
        "bq": np.ascontiguousarray(bqp.reshape(NM, 128).T),
        "ba": np.ascontiguousarray(inputs["ba"], np.float32).reshape(G, 1),
        "wqf": np.ascontiguousarray(
            wqf.reshape(16, 128, 8, 256).transpose(2, 1, 0, 3).reshape(8, 128, NK * 256)),
        "bqf": np.ascontiguousarray(inputs["bqf"], np.float32).astype(bf).reshape(1, DH),
        "wf": np.ascontiguousarray(
            wf.reshape(G, 16, 128, DHG).transpose(0, 2, 1, 3).reshape(G, 128, NK * DHG)),
        "bf": np.ascontiguousarray(inputs["bf"], np.float32).astype(bf).reshape(1, DH),
        "wc": np.ascontiguousarray(
            wc.reshape(16, 128, NJC, 500).transpose(2, 1, 0, 3).reshape(NJC, 128, NK * 500)),
        "bc": np.ascontiguousarray(inputs["bc"], np.float32).astype(bf).reshape(1, NANS),
    }
    in_maps = []
    for c in range(NCORES):
        vs = iv[c * BPC:(c + 1) * BPC].astype(bf)          # [16, DV, S]
        qs = xq[c * BPC:(c + 1) * BPC].astype(bf)          # [16, DQ]
        m = dict(shared)
        m["qT"] = np.ascontiguousarray(
            qs.T.reshape(NK, 128, BPC).transpose(1, 0, 2).reshape(128, NK * BPC))
        m["v"] = np.ascontiguousarray(
            vs.reshape(NPAIR, 2, NK, 128, S).transpose(0, 3, 2, 1, 4)
            .reshape(NPAIR, 128, NK * S2))
        m["vT"] = np.ascontiguousarray(
            vs.transpose(0, 2, 1).reshape(NPAIR, 2, 2, SH, DV)
            .transpose(0, 3, 1, 2, 4).reshape(NPAIR, SH, 4 * DV))
        in_maps.append(m)
    return in_maps


def kernel(**inputs) -> np.ndarray:
    nc = _get_module(1)
    in_maps = make_in_maps(inputs)
    res = run_bass_kernel_spmd(nc, in_maps, core_ids=list(range(NCORES)))
    return np.concatenate([res.results[c]["out"] for c in range(NCORES)], axis=0)
